# revision 7
# baseline (speedup 1.0000x reference)
"""Trainium2 Bass kernel for nn_GatedShortBlock (gated depthwise-conv block), v4 (bf16).

Math (per batch b):
  BCx = x @ w1.T ; Bg, Cg, Xg = split(BCx, 3)
  gated = Bg * Xg
  conv  = causal depthwise conv1d(gated, conv_w, K=4)  (left pad 3)
  out   = (Cg * conv) @ w2.T

Sharding: data-parallel over (batch, seq-half) -> 8 shards of 2048 tokens.
Each core computes its shard fully on-device in channel-major layout; the
3-token causal halo of `gated` at each shard start is host-computed.

All matmuls in bf16 with small low-address weight tiles. Perf structure:
  - weights host-packed tile-contiguous: every weight DMA is a [128, 512]
    row-block with 2KB contiguous per partition (2KB DMA packets run at
    ~21GB/s/engine vs 11.6 for the 512B gathers the old layout produced).
  - x ships as bf16 (halves the startup-critical DMA) and is upconverted
    to f32r on the DVE before the matmuls.
  - output stores issue on the Activation-engine DGE queue so block b+1's
    loads are not serialized behind block b's stores on the sync queue.
  - conv weights / gated halo packed into single small tiles (one DMA each).
"""

import sys

sys.path.insert(0, "/opt/trn_rl_repo")

import numpy as np
import ml_dtypes
from contextlib import ExitStack

import concourse.bass as bass
import concourse.tile as tile
from concourse import bacc, mybir
from concourse.bass_utils import run_bass_kernel_spmd

F32 = mybir.dt.float32
F32R = mybir.dt.float32r
BF16 = mybir.dt.bfloat16
NPBF16 = ml_dtypes.bfloat16
KS = 4  # conv kernel size
KG = 4  # k-subtiles per weight DMA tile
D = 2048
T = 2048  # tokens per core
TBLK = 1024
CH = 512
NWARM = 20  # PE warmup matmuls issued before the first data-dependent matmul


def build_program():
    ND = D // 128  # contraction tiles
    NC = D // 128  # channel/output tiles
    NE = 3 * NC  # w1 output tiles: [Bg: 0..15, Cg: 16..31, Xg: 32..47]
    NBLK = T // TBLK
    NCH = TBLK // CH
    NG = ND // KG  # weight DMA tiles per e-tile

    nc = bacc.Bacc(None)
    xTb = nc.dram_tensor("xTb", [D, T], BF16, kind="ExternalInput")
    # w1P tile (e,g) at rows (e*NG+g)*128, [p, ks*128+m] = w1[e*128+m, (g*KG+ks)*128+p]
    w1P = nc.dram_tensor("w1P", [NE * NG * 128, KG * 128], BF16, kind="ExternalInput")
    w2P = nc.dram_tensor("w2P", [NC * NG * 128, KG * 128], BF16, kind="ExternalInput")
    cw2 = nc.dram_tensor("cw2", [128, NC * KS], F32, kind="ExternalInput")
    gh2 = nc.dram_tensor("gh2", [128, NC * (KS - 1)], F32, kind="ExternalInput")
    outT = nc.dram_tensor("outT", [D, T], BF16, kind="ExternalOutput")

    with tile.TileContext(nc) as tc, ExitStack() as ctx:
        # weight pools FIRST: LDWEIGHTS SBUF reads get slower the higher the
        # weight tile's SBUF address (measured: +13ns/matmul at 17KB,
        # +46ns at 85KB, +60ns at 107KB) — keep weights at low addresses.
        wp = ctx.enter_context(tc.tile_pool(name="wp", bufs=12))
        w2p = ctx.enter_context(tc.tile_pool(name="w2p", bufs=6))
        xp = ctx.enter_context(tc.tile_pool(name="xp", bufs=1))
        gwp = ctx.enter_context(tc.tile_pool(name="gwp", bufs=NC))
        rp = ctx.enter_context(tc.tile_pool(name="rp", bufs=NC))
        scrp = ctx.enter_context(tc.tile_pool(name="scrp", bufs=3))
        stgp = ctx.enter_context(tc.tile_pool(name="stgp", bufs=3))
        smallp = ctx.enter_context(tc.tile_pool(name="smallp", bufs=1))
        psp = ctx.enter_context(tc.tile_pool(name="psp", bufs=8, space="PSUM"))

        cwt = smallp.tile([128, NC * KS], F32, tag="cw", name="cw")
        ghS = smallp.tile([128, NC * (KS - 1)], F32, tag="gh", name="gh")
        ghsb = [
            smallp.tile([128, KS - 1], F32, tag=f"ghc{c}", name=f"ghc{c}")
            for c in range(NC)
        ]

        # PE warmup: the HAM clock gate keeps the PE at 1.2 GHz until it has
        # been busy ~3.4us. Burn dummy matmuls on a memset tile while the
        # first x/weight DMAs are in flight so the real stream starts warm.
        jt = smallp.tile([128, 512], BF16, tag="junk", name="junk")
        nc.gpsimd.memset(jt[:], 0.0)
        psw = psp.tile([128, 512], F32, tag="ps", name="wup_ps")
        for i in range(NWARM):
            nc.tensor.matmul(psw[:], jt[:, 0:128], jt[:], start=True, stop=True)

        def load_w(w, e, b, tag="w1"):
            pool = wp if tag == "w1" else w2p
            tiles = []
            for g in range(NG):
                wt = pool.tile([128, KG * 128], BF16, tag=tag, name=f"{tag}_{b}_{e}_{g}")
                nc.sync.dma_start(
                    wt[:], w[(e * NG + g) * 128 : (e * NG + g + 1) * 128, :]
                )
                tiles.append(wt)
            return tiles

        def mm_accum(pss, wtiles, xt):
            for g in range(NG):
                for ks in range(KG):
                    k = g * KG + ks
                    w_ap = wtiles[g][:, ks * 128 : (ks + 1) * 128]
                    for u in range(NCH):
                        nc.tensor.matmul(
                            pss[u][:],
                            w_ap,
                            xt[k][:, u * CH : (u + 1) * CH],
                            start=(k == 0),
                            stop=(k == ND - 1),
                        )

        for b in range(NBLK):
            with nc.named_scope(f"blk{b}"):
                # ---- x loads (bf16, consumed directly by the PE) ----
                # Block 0: x alternates between the sync and scalar DGE
                # queues (scalar is idle at startup) so the first tiles land
                # ~2x sooner; the first tile is split so matmul 0 only waits
                # on a half-tile.
                xt = []
                for k in range(ND):
                    t = xp.tile([128, TBLK], BF16, tag=f"x{k}", name=f"x{k}_{b}")
                    if b == 0 and k == 0:
                        nc.scalar.dma_start(
                            t[:, 0:CH], xTb[k * 128 : (k + 1) * 128, 0:CH]
                        )
                        nc.sync.dma_start(
                            t[:, CH:TBLK], xTb[k * 128 : (k + 1) * 128, CH:TBLK]
                        )
                    else:
                        eng = nc.scalar if (b == 0 and k % 2 == 0) else nc.sync
                        eng.dma_start(
                            t[:], xTb[k * 128 : (k + 1) * 128, b * TBLK : (b + 1) * TBLK]
                        )
                    xt.append(t)
                    if b == 0 and k == 0:
                        # first Bg weight tiles + small tables ahead of x bulk
                        waB0 = load_w(w1P, 0, b)
                        nc.sync.dma_start(ghS[:], gh2[:, :])
                        nc.sync.dma_start(cwt[:], cw2[:, :])
                    if b == 0 and k == 3:
                        waX0 = load_w(w1P, 2 * NC, b)

                # ---- phase A: Bg, Xg -> gated ----
                gwork = []
                for c in range(NC):
                    wB = waB0 if (b == 0 and c == 0) else load_w(w1P, c, b)
                    gw = gwp.tile(
                        [128, TBLK + KS - 1], F32, tag="gw", name=f"gw{b}_{c}"
                    )
                    gwork.append(gw)
                    if b == 0:
                        nc.vector.tensor_copy(
                            gw[:, 0 : KS - 1],
                            ghS[:, c * (KS - 1) : (c + 1) * (KS - 1)],
                        )
                    else:
                        nc.vector.tensor_copy(gw[:, 0 : KS - 1], ghsb[c][:])
                    psB = [
                        psp.tile([128, CH], F32, tag="ps", name=f"psB{b}_{c}_{u}")
                        for u in range(NCH)
                    ]
                    mm_accum(psB, wB, xt)
                    wX = waX0 if (b == 0 and c == 0) else load_w(w1P, 2 * NC + c, b)
                    psX = [
                        psp.tile([128, CH], F32, tag="ps", name=f"psX{b}_{c}_{u}")
                        for u in range(NCH)
                    ]
                    mm_accum(psX, wX, xt)
                    for u in range(NCH):
                        # DVE reads at most one PSUM operand per instruction:
                        # stage Bg into gwork, then multiply Xg in place.
                        dst = gw[:, KS - 1 + u * CH : KS - 1 + (u + 1) * CH]
                        nc.vector.tensor_copy(dst, psB[u][:])
                        nc.vector.tensor_mul(dst, dst, psX[u][:])

                # ---- phase B: Cg, conv -> R ----
                Rt = []
                for c in range(NC):
                    wC = load_w(w1P, NC + c, b)
                    psC = [
                        psp.tile([128, CH], F32, tag="ps", name=f"psC{b}_{c}_{u}")
                        for u in range(NCH)
                    ]
                    mm_accum(psC, wC, xt)
                    gw = gwork[c]
                    s = scrp.tile([128, TBLK], F32, tag="scr", name=f"s0_{b}_{c}")
                    nc.vector.tensor_scalar_mul(
                        s[:], gw[:, 0:TBLK], cwt[:, c * KS : c * KS + 1]
                    )
                    for j in range(1, KS):
                        s2 = scrp.tile([128, TBLK], F32, tag="scr", name=f"s{j}_{b}_{c}")
                        nc.vector.scalar_tensor_tensor(
                            s2[:],
                            gw[:, j : j + TBLK],
                            cwt[:, c * KS + j : c * KS + j + 1],
                            s[:],
                            mybir.AluOpType.mult,
                            mybir.AluOpType.add,
                        )
                        s = s2
                    if b < NBLK - 1:
                        nc.vector.tensor_copy(ghsb[c][:], gw[:, TBLK : TBLK + KS - 1])
                    R = rp.tile([128, TBLK], BF16, tag="r", name=f"R{b}_{c}")
                    Rt.append(R)
                    for u in range(NCH):
                        nc.vector.tensor_mul(
                            R[:, u * CH : (u + 1) * CH],
                            s[:, u * CH : (u + 1) * CH],
                            psC[u][:],
                        )

                # ---- mm2: out = R.T @ w2.T (channel-major) ----
                for f in range(NC):
                    w2t = load_w(w2P, f, b, tag="w2")
                    ps2 = [
                        psp.tile([128, CH], F32, tag="ps", name=f"ps2{b}_{f}_{u}")
                        for u in range(NCH)
                    ]
                    for g in range(NG):
                        for cs in range(KG):
                            c = g * KG + cs
                            w_ap = w2t[g][:, cs * 128 : (cs + 1) * 128]
                            for u in range(NCH):
                                nc.tensor.matmul(
                                    ps2[u][:],
                                    w_ap,
                                    Rt[c][:, u * CH : (u + 1) * CH],
                                    start=(c == 0),
                                    stop=(c == NC - 1),
                                )
                    last = b == NBLK - 1
                    for u in range(NCH):
                        st = stgp.tile([128, CH], BF16, tag="stg", name=f"st{b}_{f}_{u}")
                        nc.vector.tensor_copy(st[:], ps2[u][:])
                        dst = outT[
                            f * 128 : (f + 1) * 128,
                            b * TBLK + u * CH : b * TBLK + (u + 1) * CH,
                        ]
                        if last and f >= NC - 2:
                            # tail: split the final stores across both DGE
                            # queues so the post-compute drain is short
                            h = CH // 2
                            nc.scalar.dma_start(dst[:, 0:h], st[:, 0:h])
                            nc.sync.dma_start(dst[:, h:CH], st[:, h:CH])
                        else:
                            eng = nc.sync if (last and (f * NCH + u) % 2) else nc.scalar
                            eng.dma_start(dst, st[:])

    nc.finalize()
    return nc


def _pack_w(w, n_etiles):
    """[n_etiles*128, D] -> packed [(e*NG+g)*128 + p, ks*128+m] =
    w[e*128+m, (g*KG+ks)*128+p], contiguous per [128, 512] tile."""
    ng = (D // 128) // KG
    return np.ascontiguousarray(
        w.reshape(n_etiles, 128, ng, KG, 128)
        .transpose(0, 2, 4, 3, 1)
        .reshape(n_etiles * ng * 128, KG * 128)
    ).astype(NPBF16)


def shard_inputs(x, w1, w2, conv_w):
    B, S, _ = x.shape
    NC = D // 128
    n_shards = (B * S) // T
    w1Pk = _pack_w(w1, 3 * NC)
    w2Pk = _pack_w(w2, NC)
    cw2 = np.ascontiguousarray(
        conv_w[:, 0, :].reshape(NC, 128, KS).transpose(1, 0, 2).reshape(128, NC * KS)
    ).astype(np.float32)

    shards_per_batch = S // T
    in_maps = []
    for s in range(n_shards):
        b, h = divmod(s, shards_per_batch)
        xs = x[b, h * T : (h + 1) * T, :]
        xTs = np.ascontiguousarray(xs.T).astype(NPBF16)
        if h == 0:
            gh2 = np.zeros((128, NC * (KS - 1)), np.float32)
        else:
            xh = x[b, h * T - (KS - 1) : h * T, :].astype(NPBF16).astype(np.float32)
            Bg = xh @ w1[0:D].T
            Xg = xh @ w1[2 * D : 3 * D].T
            ghs = np.ascontiguousarray((Bg * Xg).T).astype(np.float32)  # [D, 3]
            gh2 = np.ascontiguousarray(
                ghs.reshape(NC, 128, KS - 1).transpose(1, 0, 2).reshape(
                    128, NC * (KS - 1)
                )
            )
        in_maps.append(
            {"xTb": xTs, "w1P": w1Pk, "w2P": w2Pk, "cw2": cw2, "gh2": gh2}
        )
    return in_maps


_PROGRAM_CACHE = {}


def run(x, w1, w2, conv_w, trace=False):
    B, S, _ = x.shape
    if "p" not in _PROGRAM_CACHE:
        _PROGRAM_CACHE["p"] = build_program()
    nc = _PROGRAM_CACHE["p"]
    in_maps = shard_inputs(x, w1, w2, conv_w)
    n_shards = len(in_maps)
    res = run_bass_kernel_spmd(nc, in_maps, core_ids=list(range(n_shards)), trace=trace)
    shards_per_batch = S // T
    out = np.empty((B, S, D), np.float32)
    for s in range(n_shards):
        b, h = divmod(s, shards_per_batch)
        out[b, h * T : (h + 1) * T, :] = res.results[s]["outT"].T.astype(np.float32)
    return out, res


def kernel(x, w1, w2, conv_w):
    x = np.asarray(x, np.float32)
    w1 = np.asarray(w1, np.float32)
    w2 = np.asarray(w2, np.float32)
    conv_w = np.asarray(conv_w, np.float32)
    out, _ = run(x, w1, w2, conv_w)
    return out



# revision 9
# speedup vs baseline: 1.0055x; 1.0055x over previous
"""Trainium2 Bass kernel for nn_GatedShortBlock (gated depthwise-conv block), v4 (bf16).

Math (per batch b):
  BCx = x @ w1.T ; Bg, Cg, Xg = split(BCx, 3)
  gated = Bg * Xg
  conv  = causal depthwise conv1d(gated, conv_w, K=4)  (left pad 3)
  out   = (Cg * conv) @ w2.T

Sharding: data-parallel over (batch, seq-half) -> 8 shards of 2048 tokens.
Each core computes its shard fully on-device in channel-major layout; the
3-token causal halo of `gated` at each shard start is host-computed.

All matmuls in bf16 with small low-address weight tiles. Perf structure:
  - weights host-packed tile-contiguous: every weight DMA is a [128, 512]
    row-block with 2KB contiguous per partition (2KB DMA packets run at
    ~21GB/s/engine vs 11.6 for the 512B gathers the old layout produced).
  - x ships as bf16 (halves the startup-critical DMA) and is upconverted
    to f32r on the DVE before the matmuls.
  - output stores issue on the Activation-engine DGE queue so block b+1's
    loads are not serialized behind block b's stores on the sync queue.
  - conv weights / gated halo packed into single small tiles (one DMA each).
"""

import sys

sys.path.insert(0, "/opt/trn_rl_repo")

import numpy as np
import ml_dtypes
from contextlib import ExitStack

import concourse.bass as bass
import concourse.tile as tile
from concourse import bacc, mybir
from concourse.bass_utils import run_bass_kernel_spmd

F32 = mybir.dt.float32
F32R = mybir.dt.float32r
BF16 = mybir.dt.bfloat16
NPBF16 = ml_dtypes.bfloat16
KS = 4  # conv kernel size
KG = 4  # k-subtiles per weight DMA tile
D = 2048
T = 2048  # tokens per core
TBLK = 1024
CH = 512
NWARM = 6  # PE warmup matmuls issued before the first data-dependent matmul


def build_program():
    ND = D // 128  # contraction tiles
    NC = D // 128  # channel/output tiles
    NE = 3 * NC  # w1 output tiles: [Bg: 0..15, Cg: 16..31, Xg: 32..47]
    NBLK = T // TBLK
    NCH = TBLK // CH
    NG = ND // KG  # weight DMA tiles per e-tile

    nc = bacc.Bacc(None)
    xTb = nc.dram_tensor("xTb", [D, T], BF16, kind="ExternalInput")
    # w1P tile (e,g) at rows (e*NG+g)*128, [p, ks*128+m] = w1[e*128+m, (g*KG+ks)*128+p]
    w1P = nc.dram_tensor("w1P", [NE * NG * 128, KG * 128], BF16, kind="ExternalInput")
    w2P = nc.dram_tensor("w2P", [NC * NG * 128, KG * 128], BF16, kind="ExternalInput")
    cw2 = nc.dram_tensor("cw2", [128, NC * KS], F32, kind="ExternalInput")
    gh2 = nc.dram_tensor("gh2", [128, NC * (KS - 1)], F32, kind="ExternalInput")
    outT = nc.dram_tensor("outT", [D, T], BF16, kind="ExternalOutput")

    with tile.TileContext(nc) as tc, ExitStack() as ctx:
        # weight pools FIRST: LDWEIGHTS SBUF reads get slower the higher the
        # weight tile's SBUF address (measured: +13ns/matmul at 17KB,
        # +46ns at 85KB, +60ns at 107KB) — keep weights at low addresses.
        wp = ctx.enter_context(tc.tile_pool(name="wp", bufs=12))
        w2p = ctx.enter_context(tc.tile_pool(name="w2p", bufs=6))
        xp = ctx.enter_context(tc.tile_pool(name="xp", bufs=1))
        gwp = ctx.enter_context(tc.tile_pool(name="gwp", bufs=NC))
        rp = ctx.enter_context(tc.tile_pool(name="rp", bufs=NC))
        scrp = ctx.enter_context(tc.tile_pool(name="scrp", bufs=3))
        stgp = ctx.enter_context(tc.tile_pool(name="stgp", bufs=3))
        smallp = ctx.enter_context(tc.tile_pool(name="smallp", bufs=1))
        psp = ctx.enter_context(tc.tile_pool(name="psp", bufs=8, space="PSUM"))

        cwt = smallp.tile([128, NC * KS], F32, tag="cw", name="cw")
        ghS = smallp.tile([128, NC * (KS - 1)], F32, tag="gh", name="gh")
        ghsb = [
            smallp.tile([128, KS - 1], F32, tag=f"ghc{c}", name=f"ghc{c}")
            for c in range(NC)
        ]

        # PE warmup: the HAM clock gate keeps the PE at 1.2 GHz until it has
        # been busy ~3.4us. Burn dummy matmuls on a memset tile while the
        # first x/weight DMAs are in flight so the real stream starts warm.
        jt = smallp.tile([128, 512], BF16, tag="junk", name="junk")
        nc.gpsimd.memset(jt[:], 0.0)
        psw = psp.tile([128, 512], F32, tag="ps", name="wup_ps")
        for i in range(NWARM):
            nc.tensor.matmul(psw[:], jt[:, 0:128], jt[:], start=True, stop=True)

        def load_w(w, e, b, tag="w1"):
            pool = wp if tag == "w1" else w2p
            tiles = []
            for g in range(NG):
                wt = pool.tile([128, KG * 128], BF16, tag=tag, name=f"{tag}_{b}_{e}_{g}")
                nc.sync.dma_start(
                    wt[:], w[(e * NG + g) * 128 : (e * NG + g + 1) * 128, :]
                )
                tiles.append(wt)
            return tiles

        def mm_accum(pss, wtiles, xt):
            for g in range(NG):
                for ks in range(KG):
                    k = g * KG + ks
                    w_ap = wtiles[g][:, ks * 128 : (ks + 1) * 128]
                    for u in range(NCH):
                        nc.tensor.matmul(
                            pss[u][:],
                            w_ap,
                            xt[k][:, u * CH : (u + 1) * CH],
                            start=(k == 0),
                            stop=(k == ND - 1),
                        )

        for b in range(NBLK):
            with nc.named_scope(f"blk{b}"):
                # ---- x loads (bf16, consumed directly by the PE) ----
                # Block 0: x alternates between the sync and scalar DGE
                # queues (scalar is idle at startup) so the first tiles land
                # ~2x sooner; the first tile is split so matmul 0 only waits
                # on a half-tile.
                xt = []
                for k in range(ND):
                    t = xp.tile([128, TBLK], BF16, tag=f"x{k}", name=f"x{k}_{b}")
                    if b == 0 and k == 0:
                        # split so matmul 0 only waits on a half tile
                        nc.sync.dma_start(t[:, 0:CH], xTb[0:128, 0:CH])
                        nc.sync.dma_start(t[:, CH:TBLK], xTb[0:128, CH:TBLK])
                    else:
                        nc.sync.dma_start(
                            t[:], xTb[k * 128 : (k + 1) * 128, b * TBLK : (b + 1) * TBLK]
                        )
                    xt.append(t)
                    if b == 0 and k == 0:
                        # first Bg weight tiles on sync; small tables off to
                        # the idle scalar queue
                        waB0 = load_w(w1P, 0, b)
                        nc.scalar.dma_start(ghS[:], gh2[:, :])
                        nc.scalar.dma_start(cwt[:], cw2[:, :])
                    if b == 0 and k == 3:
                        waX0 = load_w(w1P, 2 * NC, b)

                # ---- phase A: Bg, Xg -> gated ----
                gwork = []
                for c in range(NC):
                    wB = waB0 if (b == 0 and c == 0) else load_w(w1P, c, b)
                    gw = gwp.tile(
                        [128, TBLK + KS - 1], F32, tag="gw", name=f"gw{b}_{c}"
                    )
                    gwork.append(gw)
                    if b == 0:
                        nc.vector.tensor_copy(
                            gw[:, 0 : KS - 1],
                            ghS[:, c * (KS - 1) : (c + 1) * (KS - 1)],
                        )
                    else:
                        nc.vector.tensor_copy(gw[:, 0 : KS - 1], ghsb[c][:])
                    psB = [
                        psp.tile([128, CH], F32, tag="ps", name=f"psB{b}_{c}_{u}")
                        for u in range(NCH)
                    ]
                    mm_accum(psB, wB, xt)
                    wX = waX0 if (b == 0 and c == 0) else load_w(w1P, 2 * NC + c, b)
                    psX = [
                        psp.tile([128, CH], F32, tag="ps", name=f"psX{b}_{c}_{u}")
                        for u in range(NCH)
                    ]
                    mm_accum(psX, wX, xt)
                    for u in range(NCH):
                        # DVE reads at most one PSUM operand per instruction:
                        # stage Bg into gwork, then multiply Xg in place.
                        dst = gw[:, KS - 1 + u * CH : KS - 1 + (u + 1) * CH]
                        nc.vector.tensor_copy(dst, psB[u][:])
                        nc.vector.tensor_mul(dst, dst, psX[u][:])

                # ---- phase B: Cg, conv -> R ----
                Rt = []
                for c in range(NC):
                    wC = load_w(w1P, NC + c, b)
                    psC = [
                        psp.tile([128, CH], F32, tag="ps", name=f"psC{b}_{c}_{u}")
                        for u in range(NCH)
                    ]
                    mm_accum(psC, wC, xt)
                    gw = gwork[c]
                    s = scrp.tile([128, TBLK], F32, tag="scr", name=f"s0_{b}_{c}")
                    nc.vector.tensor_scalar_mul(
                        s[:], gw[:, 0:TBLK], cwt[:, c * KS : c * KS + 1]
                    )
                    for j in range(1, KS):
                        s2 = scrp.tile([128, TBLK], F32, tag="scr", name=f"s{j}_{b}_{c}")
                        nc.vector.scalar_tensor_tensor(
                            s2[:],
                            gw[:, j : j + TBLK],
                            cwt[:, c * KS + j : c * KS + j + 1],
                            s[:],
                            mybir.AluOpType.mult,
                            mybir.AluOpType.add,
                        )
                        s = s2
                    if b < NBLK - 1:
                        nc.vector.tensor_copy(ghsb[c][:], gw[:, TBLK : TBLK + KS - 1])
                    R = rp.tile([128, TBLK], BF16, tag="r", name=f"R{b}_{c}")
                    Rt.append(R)
                    for u in range(NCH):
                        nc.vector.tensor_mul(
                            R[:, u * CH : (u + 1) * CH],
                            s[:, u * CH : (u + 1) * CH],
                            psC[u][:],
                        )

                # ---- mm2: out = R.T @ w2.T (channel-major) ----
                for f in range(NC):
                    w2t = load_w(w2P, f, b, tag="w2")
                    ps2 = [
                        psp.tile([128, CH], F32, tag="ps", name=f"ps2{b}_{f}_{u}")
                        for u in range(NCH)
                    ]
                    for g in range(NG):
                        for cs in range(KG):
                            c = g * KG + cs
                            w_ap = w2t[g][:, cs * 128 : (cs + 1) * 128]
                            for u in range(NCH):
                                nc.tensor.matmul(
                                    ps2[u][:],
                                    w_ap,
                                    Rt[c][:, u * CH : (u + 1) * CH],
                                    start=(c == 0),
                                    stop=(c == NC - 1),
                                )
                    last = b == NBLK - 1
                    for u in range(NCH):
                        st = stgp.tile([128, CH], BF16, tag="stg", name=f"st{b}_{f}_{u}")
                        nc.vector.tensor_copy(st[:], ps2[u][:])
                        dst = outT[
                            f * 128 : (f + 1) * 128,
                            b * TBLK + u * CH : b * TBLK + (u + 1) * CH,
                        ]
                        if last and f >= NC - 2:
                            # tail: split the final stores across both DGE
                            # queues so the post-compute drain is short
                            h = CH // 2
                            nc.scalar.dma_start(dst[:, 0:h], st[:, 0:h])
                            nc.sync.dma_start(dst[:, h:CH], st[:, h:CH])
                        else:
                            eng = nc.sync if (last and (f * NCH + u) % 2) else nc.scalar
                            eng.dma_start(dst, st[:])

    nc.finalize()
    return nc


def _pack_w(w, n_etiles):
    """[n_etiles*128, D] -> packed [(e*NG+g)*128 + p, ks*128+m] =
    w[e*128+m, (g*KG+ks)*128+p], contiguous per [128, 512] tile."""
    ng = (D // 128) // KG
    return np.ascontiguousarray(
        w.reshape(n_etiles, 128, ng, KG, 128)
        .transpose(0, 2, 4, 3, 1)
        .reshape(n_etiles * ng * 128, KG * 128)
    ).astype(NPBF16)


def shard_inputs(x, w1, w2, conv_w):
    B, S, _ = x.shape
    NC = D // 128
    n_shards = (B * S) // T
    w1Pk = _pack_w(w1, 3 * NC)
    w2Pk = _pack_w(w2, NC)
    cw2 = np.ascontiguousarray(
        conv_w[:, 0, :].reshape(NC, 128, KS).transpose(1, 0, 2).reshape(128, NC * KS)
    ).astype(np.float32)

    shards_per_batch = S // T
    in_maps = []
    for s in range(n_shards):
        b, h = divmod(s, shards_per_batch)
        xs = x[b, h * T : (h + 1) * T, :]
        xTs = np.ascontiguousarray(xs.T).astype(NPBF16)
        if h == 0:
            gh2 = np.zeros((128, NC * (KS - 1)), np.float32)
        else:
            xh = x[b, h * T - (KS - 1) : h * T, :].astype(NPBF16).astype(np.float32)
            Bg = xh @ w1[0:D].T
            Xg = xh @ w1[2 * D : 3 * D].T
            ghs = np.ascontiguousarray((Bg * Xg).T).astype(np.float32)  # [D, 3]
            gh2 = np.ascontiguousarray(
                ghs.reshape(NC, 128, KS - 1).transpose(1, 0, 2).reshape(
                    128, NC * (KS - 1)
                )
            )
        in_maps.append(
            {"xTb": xTs, "w1P": w1Pk, "w2P": w2Pk, "cw2": cw2, "gh2": gh2}
        )
    return in_maps


_PROGRAM_CACHE = {}


def run(x, w1, w2, conv_w, trace=False):
    B, S, _ = x.shape
    if "p" not in _PROGRAM_CACHE:
        _PROGRAM_CACHE["p"] = build_program()
    nc = _PROGRAM_CACHE["p"]
    in_maps = shard_inputs(x, w1, w2, conv_w)
    n_shards = len(in_maps)
    res = run_bass_kernel_spmd(nc, in_maps, core_ids=list(range(n_shards)), trace=trace)
    shards_per_batch = S // T
    out = np.empty((B, S, D), np.float32)
    for s in range(n_shards):
        b, h = divmod(s, shards_per_batch)
        out[b, h * T : (h + 1) * T, :] = res.results[s]["outT"].T.astype(np.float32)
    return out, res


def kernel(x, w1, w2, conv_w):
    x = np.asarray(x, np.float32)
    w1 = np.asarray(w1, np.float32)
    w2 = np.asarray(w2, np.float32)
    conv_w = np.asarray(conv_w, np.float32)
    out, _ = run(x, w1, w2, conv_w)
    return out

